# revision 1
# baseline (speedup 1.0000x reference)
"""Causal self-attention (QK-RMSNorm + rotary + value-embed blend) on 8 trn2 cores.

Sharding: 8 cores = 4 batches x 2 head-halves (8 heads each).
Host pre-transposes/casts inputs to fp16; device computes a per-core output
partial [1024, 1024] fp32 (output projection contracted over its 512 hdim
columns); host sums the two partials per batch.

Per-core kernel structure, software-pipelined per head-pair g in 0..3:
  proj(g): q|k|v = x @ W[:, pair-cols]  (fp16 matmuls, one fp32 psum group)
  per token-half (4 t-tiles): RMS-norm scale (ACT square -> DVE reduce ->
  ln/exp rsqrt) applied on the fp16 copy, fused q+k rotary (one DVE op set
  over all 8 (tensor,head,half) groups), DRAM write + xbar transpose DMA
  chunk into QT/KT - all overlapping the remaining proj matmuls
  scores^T[kpos, q] = K^T q per (head, kt, 512-q-chunk) into a 2-bank psum
  (bufs=2 so exp(chunk) overlaps scores(chunk+1)); the diagonal 128-block
  gets an additive -C*max(0,k-q) ramp from a rank-128 mask matmul so exp
  kills the upper triangle with no elementwise mask op
  exp via ACT straight from psum (scale=0.12 fused), fp16, packed causally
  AV for pair g-1 (one-stage software pipeline so pair g's rms/rotary/
  transpose chain never waits behind pair g-1's AV divides on DVE, and the
  qk DMAs beat the y DMAs onto the SP queue): y[q,:] accumulates
  ET[kt]^T @ [V|1] over kt; softmax denominator from the appended ones
  column; divide via per-partition reciprocal + broadcast multiply;
  y written + transposed per token-half
Then out_partial = y @ woT per t-tile, stores alternating Pool/SP queues.

The PE clock p-state ramp (0.65->1.2->2.4GHz over 3us of continuous
execution) is pre-warmed with dummy matmuls during the input-DMA window.

Numerics: QK RMS-norm bounds scores (|s| <= 0.12*64), so softmax needs no
max-subtraction; rsqrt is computed as exp(-0.5*ln(x)) so ACT only ever loads
the ln/exp table set (no table thrash against the attention exp).
"""

import sys

if "/opt/trn_rl_repo" not in sys.path:
    sys.path.insert(0, "/opt/trn_rl_repo")

import numpy as np

import concourse.bass as bass
import concourse.mybir as mybir
import concourse.tile as tile
from concourse.bass import ds, ts

P = 128
B, T, D = 4, 1024, 1024
H, DH = 16, 64
H8 = 8            # heads per core
NG = H8 // 2      # head pairs
ATTN_SCALE = 0.12
EPS = 1e-6
N_CORES = 8
TT_N = T // P     # 8 t-tiles
DT_N = D // P     # 8 d-tiles (contraction)
KT_N = T // P     # 8 kpos tiles
RT_N = (H8 * DH) // P  # 4 row-tiles of Q^T/K^T
JT_N = (H8 * DH) // P  # 4 j-tiles for out-proj contraction

f16 = mybir.dt.float16
f32 = mybir.dt.float32


def split_sync_waits(nc, max_waits=1):
    """This container's walrus rejects >1 sync-wait per instruction; spill
    extra waits onto preceding NoOps on the same engine."""
    n = 0
    for fn in nc.m.functions:
        for bb in fn.blocks:
            new_insts = []
            for inst in bb.instructions:
                si = getattr(inst, "sync_info", None)
                if si is not None and si.on_wait and len(si.on_wait) > max_waits:
                    waits = list(si.on_wait)
                    extra, keep = waits[:-max_waits], waits[-max_waits:]
                    for w in extra:
                        nop = mybir.InstNoOp(
                            name=nc.get_next_instruction_name(),
                            sync_info=mybir.SyncInfo(on_wait=[w], on_update=[]),
                            bass_nofuse=True,
                            engine=inst.engine,
                        )
                        nc.register_instruction(nop)
                        new_insts.append(nop)
                        n += 1
                    si.on_wait = keep
                new_insts.append(inst)
            bb.instructions[:] = new_insts
    return n


def build_nc(reps=1):
    nc = bass.Bass()

    # xTt: block-transposed x so each token-tile chunk is one contiguous
    # descriptor per partition: xTt[tt*128+p, dt*128+j] = x[tt*128+j, dt*128+p]
    xTt = nc.declare_dram_parameter("xTt", [T, D], f16, isOutput=False)
    # per-pair contiguous fused qkv weights: [D, pair, (q|k|v)*128]
    wqkv = nc.declare_dram_parameter("wqkv", [D, NG, 384], f16, isOutput=False)
    woT = nc.declare_dram_parameter("woT", [H8 * DH, D], f16, isOutput=False)
    ve = nc.declare_dram_parameter("ve", [T, H8 * DH], f16, isOutput=False)
    # rotary tables pre-tiled to [P, TT_N*128] (partition-contiguous)
    cosd = nc.declare_dram_parameter("cosd", [P, TT_N * P], f16, isOutput=False)
    sind = nc.declare_dram_parameter("sind", [P, TT_N * P], f16, isOutput=False)
    # causal ramp-mask factors: (Am.T @ Bm)[k, q] = -C * max(0, k - q)
    amask = nc.declare_dram_parameter("amask", [P, P], f16, isOutput=False)
    bmask = nc.declare_dram_parameter("bmask", [P, P], f16, isOutput=False)
    out = nc.declare_dram_parameter("out", [T, D], f16, isOutput=True)

    xT_v = xTt.rearrange("(tt p) c -> p tt c", p=P)
    wqkv_v = wqkv.rearrange("(dt p) g r -> p dt g r", p=P)
    wo_v = woT.rearrange("(jt p) i -> p jt i", p=P)
    ve_v = ve.rearrange("(tt p) r -> p tt r", p=P)
    cos_v = cosd.rearrange("p (tt e) -> p tt e", e=P)
    sin_v = sind.rearrange("p (tt e) -> p tt e", e=P)
    out_v = out.rearrange("(tt p) i -> p tt i", p=P)

    with tile.TileContext(nc) as tc:
        import contextlib

        with contextlib.ExitStack() as ctx:
            const = ctx.enter_context(tc.tile_pool(name="const", bufs=1))
            big = ctx.enter_context(tc.tile_pool(name="big", bufs=1))

            # -------- persistent tiles --------
            xT_sb = big.tile([P, TT_N, DT_N, P], f16)  # [p, tt, dt, tok]
            wqkv_sb = const.tile([P, DT_N, NG, 384], f16)
            wo_sb = const.tile([P, JT_N, D], f16)
            ve_sb = const.tile([P, TT_N, H8 * DH], f16)
            cos_sb = const.tile([P, TT_N, P], f16)
            sin_sb = const.tile([P, TT_N, P], f16)
            am_sb = const.tile([P, P], f16)
            bm_sb = const.tile([P, P], f16)

            # ACT activation-table pre-warm: the first Square would otherwise
            # charge a 1283ns table load right when the proj drain needs ACT
            eps_sb = const.tile([P, 1], f32)
            nc.vector.memset(eps_sb[:], EPS)
            tw_sb = const.tile([P, 1], f16)
            nc.scalar.square(tw_sb[:], eps_sb[:])

            # loads split across 3 DMA queues so pair-0 proj starts ~4us:
            # SP: first wqkv half, xT tiles 1-4, rotary tables
            # ACT: xT tile 0 + masks (ACT engine idle at t=0)
            # Pool (swdge): second wqkv half, ve, xT 5-7, pair 1
            nc.sync.dma_start(
                out=wqkv_sb[:, 0:4, 0, :], in_=wqkv_v[:, 0:4, 0, :])
            nc.scalar.dma_start(out=xT_sb[:, 0], in_=xT_v[:, 0])
            nc.gpsimd.dma_start(
                out=wqkv_sb[:, 4:8, 0, :], in_=wqkv_v[:, 4:8, 0, :])
            for tt in range(1, 5):
                nc.sync.dma_start(out=xT_sb[:, tt], in_=xT_v[:, tt])
            nc.scalar.dma_start(out=am_sb[:], in_=amask[:])
            nc.scalar.dma_start(out=bm_sb[:], in_=bmask[:])
            nc.sync.dma_start(out=cos_sb[:], in_=cos_v[:])
            nc.sync.dma_start(out=sin_sb[:], in_=sin_v[:])
            # ve first half must land before blend(g0,tt0) drains the psum
            nc.gpsimd.dma_start(out=ve_sb[:, 0:4], in_=ve_v[:, 0:4])
            nc.gpsimd.dma_start(out=ve_sb[:, 4:8], in_=ve_v[:, 4:8])
            for tt in range(5, TT_N):
                nc.gpsimd.dma_start(out=xT_sb[:, tt], in_=xT_v[:, tt])
            # pair-1 weights now; pairs 2-3 and wo are deferred into the
            # stage loop so the Pool queue is free for the rms scale ops
            nc.gpsimd.dma_start(
                out=wqkv_sb[:, :, 1, :], in_=wqkv_v[:, :, 1, :])

            vp = big.tile([P, KT_N, H8, DH + 1], f16)  # V blended + ones col
            y16 = big.tile([P, NG, TT_N, P], f16)
            QT_sb = big.tile([P, RT_N, T], f16)      # [r, t] fp16
            KT_sb = big.tile([P, RT_N, T], f16)
            yT_sb = big.tile([P, JT_N, T], f16)

            warm_sb = const.tile([P, 384], f16)
            nc.vector.memset(warm_sb[:], 0.0)
            nc.vector.memset(vp[:, :, :, DH], 1.0)

            # ET packed causally: kt block holds q in [kt*128, T)
            etoff = [kt * T - 64 * kt * (kt - 1) for kt in range(KT_N + 1)]

            for _rep in range(reps):
                with tc.tile_pool(name="projps", bufs=2, space="PSUM") as psB, \
                     tc.tile_pool(name="stps", bufs=2, space="PSUM") as psST, \
                     tc.tile_pool(name="avps", bufs=2, space="PSUM") as psAV, \
                     tc.tile_pool(name="qk", bufs=3) as qkp, \
                     tc.tile_pool(name="sq", bufs=3) as sqp, \
                     tc.tile_pool(name="etp", bufs=2) as etp, \
                     tc.tile_pool(name="qkdr", bufs=2, space="DRAM") as qkdr, \
                     tc.tile_pool(name="stat", bufs=4) as statp:
                    pair = {}

                    if _rep == 0:
                        # PE warm-up during the input-DMA window: the clock
                        # p-state ramps 0.65->1.2->2.4GHz over 3us of
                        # continuous execution; one accumulation group in a
                        # proj-psum slot, sized to end as the inputs land.
                        wp = psB.tile([P, 384], f32, tag="pqkv")
                        NWARM = 9
                        for i in range(NWARM):
                            nc.tensor.matmul(
                                wp[:], warm_sb[:, 0:128], warm_sb[:],
                                start=(i == 0), stop=(i == NWARM - 1))

                    def start_pair(g):
                        qk_dr = qkdr.tile([T, 2 * P], f16, tag="qkdr")
                        pair[g] = dict(
                            QKg=qkp.tile([P, TT_N, 2 * P], f16, tag="qkg",
                                         name=f"QKg{g}"),
                            sqg=sqp.tile([P, TT_N, 256], f16, tag="sqg",
                                         name=f"sqg{g}"),
                            qk_dr=qk_dr,
                            qk_dr_v=qk_dr.rearrange("(tt p) r -> p tt r", p=P),
                        )

                    def emit_proj_tt(g, tt):
                        if g not in pair:
                            start_pair(g)
                        pt = pair[g]
                        QKg, sqg = pt["QKg"], pt["sqg"]
                        gc = ts(g, P)
                        pj = psB.tile([P, 384], f32, tag="pqkv")
                        # single psum accumulation group for the whole bank
                        # (q,k,v ranges interleave; per-element has_written
                        # handles first-write-overwrite)
                        for dt in range(DT_N):
                            lx = xT_sb[:, tt, dt, :]
                            nc.tensor.matmul(
                                pj[:, 0:128], lx, wqkv_sb[:, dt, g, 0:128],
                                start=(dt == 0), stop=False)
                            nc.tensor.matmul(
                                pj[:, 128:256], lx, wqkv_sb[:, dt, g, 128:256],
                                start=False, stop=False)
                            nc.tensor.matmul(
                                pj[:, 256:384], lx, wqkv_sb[:, dt, g, 256:384],
                                start=False, stop=(dt == DT_N - 1))
                        # v blend -> vp
                        nc.vector.tensor_tensor(
                            vp[:, tt, 2 * g : 2 * g + 2, 0:DH],
                            pj[:, 256:384].rearrange("p (h e) -> p h e", h=2),
                            ve_sb[:, tt, gc].rearrange("p (h e) -> p h e", h=2),
                            mybir.AluOpType.add,
                        )
                        # raw q,k copy (normalized in place per half); DVE so
                        # ACT stays free for the exps
                        nc.vector.tensor_copy(
                            out=QKg[:, tt, :], in_=pj[:, 0:256])
                        nc.scalar.square(sqg[:, tt, :], pj[:, 0:256])
                        if tt % 4 != 3:
                            return
                        # ------ per-half RMS scale + fused rotary -----------
                        hf = tt // 4
                        th = ds(4 * hf, 4)
                        ms = statp.tile([P, 4, 4], f16, tag="ms")
                        with nc.allow_low_precision(
                                reason="sumsq of 64 fp16 values"):
                            nc.vector.reduce_sum(
                                ms[:],
                                sqg[:, th].rearrange("p t (h e) -> p t h e", h=4),
                                axis=mybir.AxisListType.X,
                            )
                        lnv = statp.tile([P, 4, 4], f32, tag="lnv")
                        nc.scalar.activation(
                            lnv[:], ms[:], mybir.ActivationFunctionType.Ln,
                            bias=eps_sb[:], scale=1.0 / DH,
                        )
                        scl = statp.tile([P, 4, 4], f16, tag="scl")
                        nc.scalar.activation(
                            scl[:], lnv[:], mybir.ActivationFunctionType.Exp,
                            scale=-0.5,
                        )
                        # rms scale on the (otherwise idle) Pool engine
                        nc.gpsimd.tensor_tensor(
                            QKg[:, th].rearrange("p t (h e) -> p t h e", h=4),
                            QKg[:, th].rearrange("p t (h e) -> p t h e", h=4),
                            scl[:, :, :, None].to_broadcast((P, 4, 4, DH)),
                            mybir.AluOpType.mult,
                        )
                        # fused q+k rotary: 8 (tensor,head,half) groups of 16
                        # rotating lanes, one DVE op set
                        rot = QKg[:, th].rearrange(
                            "p t (hh eh e) -> p t hh eh e", hh=8, eh=2,
                        )[:, :, :, 0, :]  # [P, 4, 8, 16]
                        qsw = statp.tile([P, 4, 8, 16], f16, tag="qsw")
                        nc.vector.tensor_copy(
                            qsw[:, :, 0::2, :], rot[:, :, 1::2, :])
                        nc.vector.tensor_copy(
                            qsw[:, :, 1::2, :], rot[:, :, 0::2, :])
                        cosv = cos_sb[:, th].rearrange(
                            "p t (hh e) -> p t hh e", hh=8)
                        sinv = sin_sb[:, th].rearrange(
                            "p t (hh e) -> p t hh e", hh=8)
                        t1 = statp.tile([P, 4, 8, 16], f16, tag="t1")
                        nc.vector.tensor_tensor(
                            t1[:], rot, cosv, mybir.AluOpType.mult)
                        t2 = statp.tile([P, 4, 8, 16], f16, tag="t2")
                        nc.vector.tensor_tensor(
                            t2[:], qsw[:], sinv, mybir.AluOpType.mult)
                        nc.vector.tensor_tensor(
                            rot, t1[:], t2[:], mybir.AluOpType.add)
                        # ------ write half + xbar-transpose q,k -------------
                        nc.sync.dma_start(
                            out=pt["qk_dr_v"][:, th, :], in_=QKg[:, th, :])
                        hs = ds(512 * hf, 512)
                        nc.sync.dma_start_transpose(
                            QT_sb[:, g, hs], pt["qk_dr"][hs, 0:P])
                        nc.sync.dma_start_transpose(
                            KT_sb[:, g, hs], pt["qk_dr"][hs, P : 2 * P])

                    def emit_sc_chunk(g, kt, qh):
                        """one (kt, 512-q-chunk) scores matmul set + exp."""
                        if (kt, qh) == (0, 0):
                            pair[g]["ET"] = etp.tile(
                                [P, 2, etoff[KT_N]], f16, tag="et",
                                name=f"ET{g}")
                        ET = pair[g]["ET"]
                        qlo = kt * P
                        qs = max(qh * 512, qlo)
                        qe = (qh + 1) * 512
                        pst = psST.tile([P, 2, 512], f32, tag="st")
                        diag = qs == qlo
                        for hb in range(2):
                            lo, hi = hb * 64, hb * 64 + 64
                            nc.tensor.matmul(
                                pst[:, hb, 0 : qe - qs],
                                KT_sb[lo:hi, g, ts(kt, P)],
                                QT_sb[lo:hi, g, ds(qs, qe - qs)],
                                start=True, stop=not diag,
                            )
                            if diag:
                                # additive -C*max(0, k-q) ramp kills the upper
                                # triangle under exp
                                nc.tensor.matmul(
                                    pst[:, hb, 0:P], am_sb[:], bm_sb[:],
                                    start=False, stop=True,
                                )
                        nc.scalar.activation(
                            ET[:, :, ds(etoff[kt] + qs - qlo, qe - qs)],
                            pst[:, :, 0 : qe - qs],
                            mybir.ActivationFunctionType.Exp,
                            scale=ATTN_SCALE,
                        )

                    def emit_av_q2(g, q2):
                        """one AV psum group + divide + y write/transpose."""
                        if q2 == 0:
                            y_dr = qkdr.tile([T, P], f16, tag="ydr",
                                             name=f"ydr{g}")
                            pair[g]["y_dr"] = y_dr
                            pair[g]["y_dr_v"] = y_dr.rearrange(
                                "(tt p) r -> p tt r", p=P)
                        ET = pair[g]["ET"]
                        y_dr, y_dr_v = pair[g]["y_dr"], pair[g]["y_dr_v"]
                        pav = psAV.tile([P, 260], f32, tag="av")
                        mms = []
                        for sub in range(2):
                            qt = 2 * q2 + sub
                            for hb in range(2):
                                for kt in range(qt + 1):
                                    mms.append((sub, qt, hb, kt))
                        for i, (sub, qt, hb, kt) in enumerate(mms):
                            nc.tensor.matmul(
                                pav[:, ds(sub * 130 + hb * 65, 65)],
                                ET[:, hb, ds(etoff[kt] + (qt - kt) * P, P)],
                                vp[:, kt, 2 * g + hb, :],
                                start=(i == 0), stop=(i == len(mms) - 1),
                            )
                        pavv = pav.rearrange("p (s h c) -> p s h c", s=2, h=2)
                        r = statp.tile([P, 2, 2], f32, tag="r")
                        nc.vector.reciprocal(r[:], pavv[:, :, :, DH : DH + 1])
                        nc.vector.tensor_tensor(
                            y16[:, g, ds(2 * q2, 2), :].rearrange(
                                "p s (h e) -> p s h e", h=2),
                            pavv[:, :, :, 0:DH],
                            r[:, :, :, None].to_broadcast((P, 2, 2, DH)),
                            mybir.AluOpType.mult,
                        )
                        if g == NG - 1:
                            # last pair feeds the out-proj: finest chunks
                            th = ds(2 * q2, 2)
                            nc.sync.dma_start(
                                out=y_dr_v[:, th, :], in_=y16[:, g, th, :])
                            nc.sync.dma_start_transpose(
                                yT_sb[:, g, ds(256 * q2, 256)],
                                y_dr[ds(256 * q2, 256), :])
                        elif q2 % 2 == 1:
                            hf = q2 // 2
                            th = ds(4 * hf, 4)
                            nc.sync.dma_start(
                                out=y_dr_v[:, th, :], in_=y16[:, g, th, :])
                            nc.sync.dma_start_transpose(
                                yT_sb[:, g, ds(512 * hf, 512)],
                                y_dr[ds(512 * hf, 512), :])

                    # ======== software-pipelined stage schedule =============
                    # stage g emits pair g's scores/exp chunks explicitly
                    # interleaved with later pairs' proj quanta and earlier
                    # pairs' AV quanta, so the PE always has ~1.3us of filler
                    # between score chunks while ACT catches up on the exps.
                    # proj runs 1.5 stages ahead so pair-3's rms/rotary/
                    # transpose chain completes well before stage 3 reads it.
                    def chunks_of(g):
                        return ([("sc", g, kt, 0) for kt in range(4)]
                                + [("sc", g, kt, 1) for kt in range(4)]
                                + [("sc", g, kt, 1) for kt in range(4, KT_N)])

                    def interleave(chunks, fillers):
                        seq = []
                        for i, c in enumerate(chunks):
                            seq.append(c)
                            if i < len(fillers):
                                seq.append(fillers[i])
                        return seq + fillers[len(chunks):]

                    for tt in range(TT_N):
                        emit_proj_tt(0, tt)

                    stages = {}
                    p = lambda g, lo, hi: [("proj", g, tt)
                                           for tt in range(lo, hi)]
                    a = lambda g: [("av", g, q2) for q2 in range(4)]
                    stages[0] = interleave(
                        chunks_of(0), p(1, 0, 8) + p(2, 0, 2))
                    stages[1] = interleave(
                        chunks_of(1), p(2, 2, 8) + p(3, 0, 4) + a(0))
                    stages[2] = interleave(
                        chunks_of(2), p(3, 4, 8) + a(1))
                    c3, a2q, a3q = chunks_of(3), a(2), a(3)
                    stages[3] = [c3[0], a2q[0], c3[1], a2q[1], c3[2], a2q[2],
                                 c3[3], a2q[3], c3[4], a3q[0], c3[5], a3q[1],
                                 c3[6], c3[7], c3[8], c3[9], a3q[2], c3[10],
                                 c3[11], a3q[3]]

                    for g in range(NG):
                        # deferred weight loads keep the early Pool queue
                        # free for the rms scale ops
                        if g == 0:
                            nc.gpsimd.dma_start(
                                out=wqkv_sb[:, :, 2, :], in_=wqkv_v[:, :, 2, :])
                            nc.gpsimd.dma_start(
                                out=wqkv_sb[:, :, 3, :], in_=wqkv_v[:, :, 3, :])
                        elif g == 1:
                            nc.gpsimd.dma_start(out=wo_sb[:], in_=wo_v[:])
                        for quantum in stages[g]:
                            kind = quantum[0]
                            if kind == "sc":
                                emit_sc_chunk(quantum[1], quantum[2], quantum[3])
                            elif kind == "proj":
                                emit_proj_tt(quantum[1], quantum[2])
                            else:
                                emit_av_q2(quantum[1], quantum[2])
                # ================= output projection =====================
                # per-(tt,ic) staging + stores alternating Pool/SP queues;
                # deep psF so jt=0..2 accumulation hoists under the pair-3 tail
                with tc.tile_pool(name="outps", bufs=6, space="PSUM") as psF, \
                     tc.tile_pool(name="outstage", bufs=4) as osp:
                    for tt in range(TT_N):
                        for ic in range(2):
                            po = psF.tile([P, 512], f32, tag="po")
                            for jt in range(JT_N):
                                nc.tensor.matmul(
                                    po[:],
                                    yT_sb[:, jt, ts(tt, P)],
                                    wo_sb[:, jt, ds(ic * 512, 512)],
                                    start=(jt == 0), stop=(jt == JT_N - 1),
                                )
                            # f16 staging halves the store DMA; the partial is
                            # summed with the other core's on the host in f32
                            osb = osp.tile([P, 512], f16, tag="osb")
                            if ic == 0:
                                nc.scalar.copy(out=osb[:], in_=po[:])
                            else:
                                nc.vector.tensor_copy(out=osb[:], in_=po[:])
                            eng = nc.gpsimd if ic == 0 else nc.sync
                            eng.dma_start(
                                out=out_v[:, tt, ds(ic * 512, 512)],
                                in_=osb[:])

    split_sync_waits(nc)
    return nc


def make_core_inputs(x, qkvo_w, value_embeds, lambda_v):
    """Host-side prep: returns list of per-core input dicts (fp16)."""
    x = np.asarray(x)
    qkvo_w = np.asarray(qkvo_w)
    value_embeds = np.asarray(value_embeds)
    lambda_v = np.asarray(lambda_v)

    freq = (1.0 / 1024.0) ** np.linspace(0.0, 1.0, DH // 4, dtype=np.float32)
    theta = np.arange(T, dtype=np.float32)[:, None] * freq[None, :]  # [T, 16]
    cos = np.cos(theta).astype(np.float32)
    sin = np.sin(theta).astype(np.float32)
    # [T, 128] tiled over all 8 (q|k, head, half) groups: cos repeats, sin
    # alternates sign; then re-tiled to [P, TT_N*128] (partition-contiguous)
    cos128 = np.concatenate([cos] * 8, axis=1).astype(np.float16)
    sin128 = np.concatenate([sin, -sin] * 4, axis=1).astype(np.float16)
    cosP = cos128.reshape(TT_N, P, P).transpose(1, 0, 2).reshape(P, TT_N * P)
    sinP = sin128.reshape(TT_N, P, P).transpose(1, 0, 2).reshape(P, TT_N * P)
    cosP = np.ascontiguousarray(cosP)
    sinP = np.ascontiguousarray(sinP)
    # additive causal ramp mask: (amask.T @ bmask)[k, q] = -2000*max(0, k-q)
    jj = np.arange(P)
    amask_np = (jj[None, :] >= jj[:, None]).astype(np.float16)   # [j, k]
    bmask_np = (-2000.0 * (jj[:, None] > jj[None, :])).astype(np.float16)

    in_maps = []
    for c in range(N_CORES):
        b, hh = c // 2, c % 2
        R = slice(hh * H8 * DH, (hh + 1) * H8 * DH)
        wq = qkvo_w[0][R].T  # [D, 512]
        wk = qkvo_w[1][R].T
        wv = (lambda_v[0] * qkvo_w[2][R]).T
        # [D, NG, 384]: per pair the 128 q cols, 128 k cols, 128 v cols
        wqkv = np.empty((D, NG, 384), dtype=np.float16)
        for g in range(NG):
            wqkv[:, g, 0:128] = wq[:, g * 128 : (g + 1) * 128]
            wqkv[:, g, 128:256] = wk[:, g * 128 : (g + 1) * 128]
            wqkv[:, g, 256:384] = wv[:, g * 128 : (g + 1) * 128]
        # block-transpose: xTt[tt*128+p, dt*128+j] = x[b][tt*128+j, dt*128+p]
        xb = x[b].reshape(TT_N, P, DT_N, P)
        xTt = np.ascontiguousarray(
            xb.transpose(0, 3, 2, 1).reshape(T, D)).astype(np.float16)
        in_maps.append({
            "xTt": xTt,
            "wqkv": wqkv,
            "woT": np.ascontiguousarray(qkvo_w[3][:, R].T).astype(np.float16),
            "ve": (lambda_v[1] * value_embeds[:T, R]).astype(np.float16),
            "cosd": cosP,
            "sind": sinP,
            "amask": amask_np,
            "bmask": bmask_np,
        })
    return in_maps


_NC_CACHE = {}


def _get_nc(reps=1):
    if reps not in _NC_CACHE:
        _NC_CACHE[reps] = build_nc(reps)
    return _NC_CACHE[reps]


def kernel(x, qkvo_w, value_embeds, lambda_v):
    from concourse.bass_utils import run_bass_kernel_spmd

    nc = _get_nc()
    in_maps = make_core_inputs(x, qkvo_w, value_embeds, lambda_v)
    res = run_bass_kernel_spmd(nc, in_maps, list(range(N_CORES))).results
    out = np.empty((B, T, D), dtype=np.float32)
    for b in range(B):
        out[b] = (res[2 * b]["out"].astype(np.float32)
                  + res[2 * b + 1]["out"].astype(np.float32))
    return out



# revision 52
# speedup vs baseline: 1.0666x; 1.0666x over previous
"""Causal self-attention (QK-RMSNorm + rotary + value-embed blend) on 8 trn2 cores.

Sharding: 8 cores = 4 batches x 2 head-halves (8 heads each).
Host pre-transposes/casts inputs to fp16; device computes a per-core output
partial [1024, 1024] fp32 (output projection contracted over its 512 hdim
columns); host sums the two partials per batch.

Per-core kernel structure, software-pipelined per head-pair g in 0..3:
  proj(g): q|k|v = x @ W[:, pair-cols]  (fp16 matmuls, one fp32 psum group)
  per token-half (4 t-tiles): RMS-norm scale (ACT square -> DVE reduce ->
  ln/exp rsqrt) applied on the fp16 copy, fused q+k rotary (one DVE op set
  over all 8 (tensor,head,half) groups), DRAM write + xbar transpose DMA
  chunk into QT/KT - all overlapping the remaining proj matmuls
  scores^T[kpos, q] = K^T q per (head, kt, 512-q-chunk) into a 2-bank psum
  (bufs=2 so exp(chunk) overlaps scores(chunk+1)); the diagonal 128-block
  gets an additive -C*max(0,k-q) ramp from a rank-128 mask matmul so exp
  kills the upper triangle with no elementwise mask op
  exp via ACT straight from psum (scale=0.12 fused), fp16, packed causally
  AV for pair g-1 (one-stage software pipeline so pair g's rms/rotary/
  transpose chain never waits behind pair g-1's AV divides on DVE, and the
  qk DMAs beat the y DMAs onto the SP queue): y[q,:] accumulates
  ET[kt]^T @ [V|1] over kt; softmax denominator from the appended ones
  column; divide via per-partition reciprocal + broadcast multiply;
  y written + transposed per token-half
Then out_partial = y @ woT per t-tile, stores alternating Pool/SP queues.

The PE clock p-state ramp (0.65->1.2->2.4GHz over 3us of continuous
execution) is pre-warmed with dummy matmuls during the input-DMA window.

Numerics: QK RMS-norm bounds scores (|s| <= 0.12*64), so softmax needs no
max-subtraction; rsqrt is computed as exp(-0.5*ln(x)) so ACT only ever loads
the ln/exp table set (no table thrash against the attention exp).
"""

import sys

if "/opt/trn_rl_repo" not in sys.path:
    sys.path.insert(0, "/opt/trn_rl_repo")

import numpy as np

import concourse.bass as bass
import concourse.mybir as mybir
import concourse.tile as tile
from concourse.bass import ds, ts

P = 128
B, T, D = 4, 1024, 1024
H, DH = 16, 64
H8 = 8            # heads per core
NG = H8 // 2      # head pairs
ATTN_SCALE = 0.12
EPS = 1e-6
N_CORES = 8
TT_N = T // P     # 8 t-tiles
DT_N = D // P     # 8 d-tiles (contraction)
DT2_N = DT_N // 2  # 4 double-row contraction chunks of 256
KT_N = T // P     # 8 kpos tiles
RT_N = (H8 * DH) // P  # 4 row-tiles of Q^T/K^T
JT_N = (H8 * DH) // P  # 4 j-tiles for out-proj contraction

# fp8 projection scaling: weights are stored as 64*w in fp8e4m3 (centers
# their ~0.016 rms in the e4m3 normal range); the psum q,k come out 64x and
# are rescaled by 1/8 at the psum drain so fp16 squares stay in range, with
# the remaining 1/8 absorbed by the rms ln/exp pipeline (ms is 64x true, so
# exp(-.5 ln) emits scl/8). v stays 64x through attention; woT carries 1/64.
WSCALE = 64.0
QK_DRAIN = 1.0 / 8.0

f8 = mybir.dt.float8e4
f16 = mybir.dt.float16
f32 = mybir.dt.float32
DR = mybir.MatmulPerfMode.DoubleRow


def split_sync_waits(nc, max_waits=1):
    """This container's walrus rejects >1 sync-wait per instruction; spill
    extra waits onto preceding NoOps on the same engine."""
    n = 0
    for fn in nc.m.functions:
        for bb in fn.blocks:
            new_insts = []
            for inst in bb.instructions:
                si = getattr(inst, "sync_info", None)
                if si is not None and si.on_wait and len(si.on_wait) > max_waits:
                    waits = list(si.on_wait)
                    extra, keep = waits[:-max_waits], waits[-max_waits:]
                    for w in extra:
                        nop = mybir.InstNoOp(
                            name=nc.get_next_instruction_name(),
                            sync_info=mybir.SyncInfo(on_wait=[w], on_update=[]),
                            bass_nofuse=True,
                            engine=inst.engine,
                        )
                        nc.register_instruction(nop)
                        new_insts.append(nop)
                        n += 1
                    si.on_wait = keep
                new_insts.append(inst)
            bb.instructions[:] = new_insts
    return n


def build_nc(reps=1):
    nc = bass.Bass()

    # x8h/x8l: block-transposed x, split into fp8 value + fp8 residual:
    # x8?[tt*128+p, dt*128+j] = fp8split(x[tt*128+j, dt*128+p])
    x8h = nc.declare_dram_parameter("x8h", [T, D], f8, isOutput=False)
    x8l = nc.declare_dram_parameter("x8l", [T, D], f8, isOutput=False)
    # per-pair contiguous fused qkv weights (64x-scaled fp8 value+residual):
    # [D, pair, (q|k|v)*128]
    w8h = nc.declare_dram_parameter("w8h", [D, NG, 384], f8, isOutput=False)
    w8l = nc.declare_dram_parameter("w8l", [D, NG, 384], f8, isOutput=False)
    woT = nc.declare_dram_parameter("woT", [H8 * DH, D], f16, isOutput=False)
    ve = nc.declare_dram_parameter("ve", [T, H8 * DH], f16, isOutput=False)
    # rotary tables pre-tiled to [P, TT_N*128] (partition-contiguous)
    cosd = nc.declare_dram_parameter("cosd", [P, TT_N * P], f16, isOutput=False)
    sind = nc.declare_dram_parameter("sind", [P, TT_N * P], f16, isOutput=False)
    # causal ramp-mask factors in fp8 double-row layout [64, 2, 128]:
    # (Am.T @ Bm)[k, q] = -240 * max(0, k - q)
    amask = nc.declare_dram_parameter("amask", [64, 2, P], f8, isOutput=False)
    bmask = nc.declare_dram_parameter("bmask", [64, 2, P], f8, isOutput=False)
    # identity (PE-transpose permutation operand)
    ident = nc.declare_dram_parameter("ident", [P, P], f16, isOutput=False)
    out = nc.declare_dram_parameter("out", [T, D], f16, isOutput=True)

    xh_v = x8h.rearrange("(tt p) c -> p tt c", p=P)
    xl_v = x8l.rearrange("(tt p) c -> p tt c", p=P)
    wh_v = w8h.rearrange("(dt p) g r -> p dt g r", p=P)
    wl_v = w8l.rearrange("(dt p) g r -> p dt g r", p=P)
    wo_v = woT.rearrange("(jt p) i -> p jt i", p=P)
    ve_v = ve.rearrange("(tt p) r -> p tt r", p=P)
    cos_v = cosd.rearrange("p (tt e) -> p tt e", e=P)
    sin_v = sind.rearrange("p (tt e) -> p tt e", e=P)
    out_v = out.rearrange("(tt p) i -> p tt i", p=P)

    with tile.TileContext(nc) as tc:
        import contextlib

        with contextlib.ExitStack() as ctx:
            const = ctx.enter_context(tc.tile_pool(name="const", bufs=1))
            big = ctx.enter_context(tc.tile_pool(name="big", bufs=1))

            # -------- persistent tiles --------
            xh_sb = big.tile([P, TT_N, DT_N, P], f8)  # [p, tt, dt, tok]
            xl_sb = big.tile([P, TT_N, DT_N, P], f8)
            wh_sb = const.tile([P, DT_N, NG, 384], f8)
            wl_sb = const.tile([P, DT_N, NG, 384], f8)
            wo_sb = const.tile([P, JT_N, D], f16)
            ve_sb = const.tile([P, TT_N, H8 * DH], f16)
            cos_sb = const.tile([P, TT_N, P], f16)
            sin_sb = const.tile([P, TT_N, P], f16)
            am_sb = const.tile([64, 2, P], f8)
            bm_sb = const.tile([64, 2, P], f8)
            id_sb = const.tile([P, P], f16)

            # ACT activation-table pre-warm: the first Ln would otherwise
            # charge a 1283ns table load right when the rms path needs ACT
            eps_sb = const.tile([P, 1], f32)
            nc.vector.memset(eps_sb[:], EPS * 64.0)
            tw_sb = const.tile([P, 1], f32)
            nc.scalar.activation(
                tw_sb[:], eps_sb[:], mybir.ActivationFunctionType.Ln)

            # input loads: each DMA dispatch costs engine time (SP 565ns,
            # ACT/DVE 667ns, Pool ~1us swdge descriptor gen), so urgent
            # pair-0 operands go wide across SP/ACT/DVE queues, the Pool
            # queue stays thin early (rms ops share that engine), and the
            # non-urgent remainder is re-dispatched inside the stage loop.
            nc.sync.dma_start(out=wh_sb[:, :, 0, :], in_=wh_v[:, :, 0, :])
            nc.scalar.dma_start(out=xh_sb[:, 0], in_=xh_v[:, 0])
            nc.scalar.dma_start(out=wl_sb[:, :, 0, :], in_=wl_v[:, :, 0, :])
            nc.scalar.dma_start(out=xl_sb[:, 0], in_=xl_v[:, 0])
            # batched token-tile loads: arrival tracks pair-0 proj's cadence
            nc.sync.dma_start(out=xh_sb[:, 1:3], in_=xh_v[:, 1:3])
            nc.sync.dma_start(out=xh_sb[:, 3:5], in_=xh_v[:, 3:5])
            nc.sync.dma_start(out=xh_sb[:, 5:8], in_=xh_v[:, 5:8])
            nc.scalar.dma_start(out=xl_sb[:, 1], in_=xl_v[:, 1])
            nc.scalar.dma_start(out=xl_sb[:, 4], in_=xl_v[:, 4])
            nc.scalar.dma_start(out=am_sb[:], in_=amask[:])
            nc.scalar.dma_start(out=bm_sb[:], in_=bmask[:])
            nc.scalar.dma_start(out=wl_sb[:, :, 1, :], in_=wl_v[:, :, 1, :])
            nc.sync.dma_start(out=cos_sb[:], in_=cos_v[:])
            nc.sync.dma_start(out=sin_sb[:], in_=sin_v[:])
            nc.sync.dma_start(out=wh_sb[:, :, 1, :], in_=wh_v[:, :, 1, :])
            # Pool queue (swdge gen is ~1us of Pool-engine time per DMA, so
            # few, fat transfers): xl pair first, full ve, xl tail, ident
            nc.gpsimd.dma_start(out=xl_sb[:, 2:4], in_=xl_v[:, 2:4])
            nc.gpsimd.dma_start(out=ve_sb[:], in_=ve_v[:])
            nc.gpsimd.dma_start(out=xl_sb[:, 5:8], in_=xl_v[:, 5:8])
            nc.gpsimd.dma_start(out=id_sb[:], in_=ident[:])
            deferred_loads = [
                lambda: nc.gpsimd.dma_start(
                    out=wh_sb[:, :, 2, :], in_=wh_v[:, :, 2, :]),
                lambda: nc.gpsimd.dma_start(
                    out=wl_sb[:, :, 2, :], in_=wl_v[:, :, 2, :]),
                lambda: nc.gpsimd.dma_start(
                    out=wh_sb[:, :, 3, :], in_=wh_v[:, :, 3, :]),
                lambda: nc.gpsimd.dma_start(
                    out=wl_sb[:, :, 3, :], in_=wl_v[:, :, 3, :]),
                lambda: nc.gpsimd.dma_start(out=wo_sb[:], in_=wo_v[:]),
            ]

            vp = big.tile([P, KT_N, H8, DH + 1], f16)  # V blended + ones col
            y16 = big.tile([P, NG, TT_N, P], f16)
            QT_sb = big.tile([P, RT_N, T], f16)      # [r, t] fp16
            KT_sb = big.tile([P, RT_N, T], f16)
            yT_sb = big.tile([P, JT_N, T], f16)

            warm_sb = const.tile([P, 384], f16)
            nc.vector.memset(warm_sb[:], 0.0)
            nc.vector.memset(vp[:, :, :, DH], 1.0)

            # ET packed causally: kt block holds q in [kt*128, T)
            etoff = [kt * T - 64 * kt * (kt - 1) for kt in range(KT_N + 1)]

            for _rep in range(reps):
                with tc.tile_pool(name="projps", bufs=4, space="PSUM") as psB, \
                     tc.tile_pool(name="stps", bufs=2, space="PSUM") as psST, \
                     tc.tile_pool(name="qk", bufs=3) as qkp, \
                     tc.tile_pool(name="sq", bufs=3) as sqp, \
                     tc.tile_pool(name="etp", bufs=2) as etp, \
                     tc.tile_pool(name="qkdr", bufs=2, space="DRAM") as qkdr, \
                     tc.tile_pool(name="outstage", bufs=4) as osp, \
                     tc.tile_pool(name="stat", bufs=4) as statp:
                    pair = {}

                    if _rep == 0:
                        # PE warm-up during the input-DMA window: the clock
                        # p-state ramps 0.65->1.2->2.4GHz over 3us of
                        # continuous execution; one accumulation group in a
                        # proj-psum slot, sized to end as the inputs land.
                        wp = psB.tile([P, 384], f32, tag="pqkv")
                        NWARM = 9
                        for i in range(NWARM):
                            nc.tensor.matmul(
                                wp[:], warm_sb[:, 0:128], warm_sb[:],
                                start=(i == 0), stop=(i == NWARM - 1))

                    def start_pair(g):
                        qk_dr = qkdr.tile([T, 2 * P], f16, tag="qkdr")
                        pair[g] = dict(
                            QKg=qkp.tile([P, TT_N, 2 * P], f16, tag="qkg",
                                         name=f"QKg{g}"),
                            sqg=sqp.tile([P, TT_N, 256], f16, tag="sqg",
                                         name=f"sqg{g}"),
                            qk_dr=qk_dr,
                            qk_dr_v=qk_dr.rearrange("(tt p) r -> p tt r", p=P),
                        )

                    def emit_proj_tt(g, tt):
                        if g not in pair:
                            start_pair(g)
                        pt = pair[g]
                        QKg, sqg = pt["QKg"], pt["sqg"]
                        gc = ts(g, P)
                        pj = psB.tile([P, 384], f32, tag="pqkv")
                        # 3-term fp8 double-row proj: xh@wh + xl@wh + xh@wl
                        # (the dropped xl@wl term is ~1e-3 relative); one psum
                        # accumulation group, contraction 256 per instruction.
                        # term-major order so the hi*hi matmuls can start
                        # before the residual operands' DMAs land
                        n_mm = 3 * DT2_N
                        i = 0
                        for lx, lw in ((xh_sb, wh_sb), (xl_sb, wh_sb),
                                       (xh_sb, wl_sb)):
                            for dt2 in range(DT2_N):
                                dd = ds(2 * dt2, 2)
                                nc.tensor.matmul(
                                    pj[:], lx[:, tt, dd, :], lw[:, dd, g, :],
                                    perf_mode=DR,
                                    start=(i == 0), stop=(i == n_mm - 1))
                                i += 1
                        # v blend -> vp (DVE: GPSIMD cannot access PSUM)
                        nc.vector.tensor_tensor(
                            vp[:, tt, 2 * g : 2 * g + 2, 0:DH],
                            pj[:, 256:384].rearrange("p (h e) -> p h e", h=2),
                            ve_sb[:, tt, gc].rearrange("p (h e) -> p h e", h=2),
                            mybir.AluOpType.add,
                        )
                        # raw q,k drain at 1/8 scale (fp16 squares stay in
                        # range); DVE so ACT stays free for the exps
                        nc.vector.tensor_scalar_mul(
                            QKg[:, tt, :], pj[:, 0:256], QK_DRAIN)
                        # sumsq source on DVE (fp16 2x) instead of ACT square
                        nc.vector.tensor_tensor(
                            sqg[:, tt, :], QKg[:, tt, :], QKg[:, tt, :],
                            mybir.AluOpType.mult)
                        if tt % 4 != 3:
                            return
                        # ------ per-half RMS scale + fused rotary -----------
                        hf = tt // 4
                        th = ds(4 * hf, 4)
                        ms = statp.tile([P, 4, 4], f16, tag="ms")
                        with nc.allow_low_precision(
                                reason="sumsq of 64 fp16 values"):
                            nc.vector.reduce_sum(
                                ms[:],
                                sqg[:, th].rearrange("p t (h e) -> p t h e", h=4),
                                axis=mybir.AxisListType.X,
                            )
                        lnv = statp.tile([P, 4, 4], f32, tag="lnv")
                        nc.scalar.activation(
                            lnv[:], ms[:], mybir.ActivationFunctionType.Ln,
                            bias=eps_sb[:], scale=1.0 / DH,
                        )
                        scl = statp.tile([P, 4, 4], f16, tag="scl")
                        nc.scalar.activation(
                            scl[:], lnv[:], mybir.ActivationFunctionType.Exp,
                            scale=-0.5,
                        )
                        # rms scale on the Pool engine
                        nc.gpsimd.tensor_tensor(
                            QKg[:, th].rearrange("p t (h e) -> p t h e", h=4),
                            QKg[:, th].rearrange("p t (h e) -> p t h e", h=4),
                            scl[:, :, :, None].to_broadcast((P, 4, 4, DH)),
                            mybir.AluOpType.mult,
                        )
                        # fused q+k rotary: 8 (tensor,head,half) groups of 16
                        # rotating lanes; the partner-lane operand is a
                        # negative-stride view (pair dim reversed), saving the
                        # two explicit swap copies
                        rot = QKg[:, th].rearrange(
                            "p t (hh eh e) -> p t hh eh e", hh=8, eh=2,
                        )[:, :, :, 0, :]  # [P, 4, 8, 16]
                        rsw = QKg[:, th].rearrange(
                            "p t (h2 e2 eh e) -> p t h2 e2 eh e", h2=4, e2=2,
                            eh=2,
                        )[:, :, :, ::-1, 0, :]  # [P, 4, 4, 2, 16] swapped
                        cosv = cos_sb[:, th].rearrange(
                            "p t (hh e) -> p t hh e", hh=8)
                        sinv = sin_sb[:, th].rearrange(
                            "p t (h2 e2 e) -> p t h2 e2 e", h2=4, e2=2)
                        t1 = statp.tile([P, 4, 8, 16], f16, tag="t1")
                        nc.vector.tensor_tensor(
                            t1[:], rot, cosv, mybir.AluOpType.mult)
                        t2 = statp.tile([P, 4, 4, 2, 16], f16, tag="t2")
                        nc.vector.tensor_tensor(
                            t2[:], rsw, sinv, mybir.AluOpType.mult)
                        nc.vector.tensor_tensor(
                            rot, t1[:],
                            t2.rearrange("p t h2 e2 e -> p t (h2 e2) e"),
                            mybir.AluOpType.add)
                        # ------ write half + xbar-transpose q,k -------------
                        nc.sync.dma_start(
                            out=pt["qk_dr_v"][:, th, :], in_=QKg[:, th, :])
                        hs = ds(512 * hf, 512)
                        nc.sync.dma_start_transpose(
                            QT_sb[:, g, hs], pt["qk_dr"][hs, 0:P])
                        nc.sync.dma_start_transpose(
                            KT_sb[:, g, hs], pt["qk_dr"][hs, P : 2 * P])

                    def emit_sc_chunk(g, kt, qh):
                        """one (kt, 512-q-chunk) scores matmul set + exp."""
                        if (kt, qh) == (0, 0):
                            pair[g]["ET"] = etp.tile(
                                [P, 2, etoff[KT_N]], f16, tag="et",
                                name=f"ET{g}")
                        ET = pair[g]["ET"]
                        qlo = kt * P
                        qs = max(qh * 512, qlo)
                        qe = (qh + 1) * 512
                        pst = psST.tile([P, 2, 512], f32, tag="st")
                        diag = qs == qlo
                        for hb in range(2):
                            lo, hi = hb * 64, hb * 64 + 64
                            nc.tensor.matmul(
                                pst[:, hb, 0 : qe - qs],
                                KT_sb[lo:hi, g, ts(kt, P)],
                                QT_sb[lo:hi, g, ds(qs, qe - qs)],
                                start=True, stop=not diag,
                            )
                            if diag:
                                # additive -C*max(0, k-q) ramp kills the upper
                                # triangle under exp (fp8 double-row, C=240:
                                # slope C*ATTN_SCALE = 28.8 per step)
                                nc.tensor.matmul(
                                    pst[:, hb, 0:P], am_sb[:], bm_sb[:],
                                    perf_mode=DR, start=False, stop=True,
                                )
                        nc.scalar.activation(
                            ET[:, :, ds(etoff[kt] + qs - qlo, qe - qs)],
                            pst[:, :, 0 : qe - qs],
                            mybir.ActivationFunctionType.Exp,
                            scale=ATTN_SCALE,
                        )

                    def emit_av_q2(g, q2):
                        """one AV psum group + divide + y write/transpose."""
                        if q2 == 0:
                            y_dr = qkdr.tile([T, P], f16, tag="ydr",
                                             name=f"ydr{g}")
                            pair[g]["y_dr"] = y_dr
                            pair[g]["y_dr_v"] = y_dr.rearrange(
                                "(tt p) r -> p tt r", p=P)
                        ET = pair[g]["ET"]
                        y_dr, y_dr_v = pair[g]["y_dr"], pair[g]["y_dr_v"]
                        # AV shares the 4-deep proj/out psum ring (1 bank)
                        pav = psB.tile([P, 260], f32, tag="pqkv")
                        mms = []
                        for sub in range(2):
                            qt = 2 * q2 + sub
                            for hb in range(2):
                                for kt in range(qt + 1):
                                    mms.append((sub, qt, hb, kt))
                        for i, (sub, qt, hb, kt) in enumerate(mms):
                            nc.tensor.matmul(
                                pav[:, ds(sub * 130 + hb * 65, 65)],
                                ET[:, hb, ds(etoff[kt] + (qt - kt) * P, P)],
                                vp[:, kt, 2 * g + hb, :],
                                start=(i == 0), stop=(i == len(mms) - 1),
                            )
                        pavv = pav.rearrange("p (s h c) -> p s h c", s=2, h=2)
                        r = statp.tile([P, 2, 2], f32, tag="r")
                        nc.vector.reciprocal(r[:], pavv[:, :, :, DH : DH + 1])
                        last_pair = g == NG - 1
                        # divide on DVE (GPSIMD cannot access PSUM)
                        nc.vector.tensor_tensor(
                            y16[:, g, ds(2 * q2, 2), :].rearrange(
                                "p s (h e) -> p s h e", h=2),
                            pavv[:, :, :, 0:DH],
                            r[:, :, :, None].to_broadcast((P, 2, 2, DH)),
                            mybir.AluOpType.mult,
                        )
                        if last_pair:
                            # pair 3 feeds the out-proj directly: PE-transpose
                            # its y (no DRAM roundtrip), cutting ~3us off each
                            # out-group's yT dependency chain
                            ptr = psB.tile([P, 2, P], f16, tag="pqkv",
                                           name=f"ptr{q2}")
                            for sub in range(2):
                                nc.tensor.matmul(
                                    ptr[:, sub, :],
                                    y16[:, g, 2 * q2 + sub, :], id_sb[:],
                                    is_transpose=True)
                            nc.vector.tensor_copy(
                                out=yT_sb[:, g, ds(256 * q2, 256)],
                                in_=ptr.rearrange("p s c -> p (s c)"))
                        elif q2 % 2 == 1:
                            hf = q2 // 2
                            th = ds(4 * hf, 4)
                            nc.sync.dma_start(
                                out=y_dr_v[:, th, :], in_=y16[:, g, th, :])
                            nc.sync.dma_start_transpose(
                                yT_sb[:, g, ds(512 * hf, 512)],
                                y_dr[ds(512 * hf, 512), :])

                    opart = {}

                    def emit_out(tt, ic, jts=None):
                        """one out-proj group: accum matmuls on the psB ring
                        + f16 staging + store. jts splits the group: a first
                        call with jts=(0,3) pre-accumulates jt 0-2, a later
                        call with jts=(3,4) finishes + drains (tail trick for
                        the groups gated on pair-3's last yT chunk)."""
                        lo, hi = jts if jts is not None else (0, JT_N)
                        if lo == 0:
                            # split groups live on the (by then idle) scores
                            # ring: the round-robin psB ring cannot hold a
                            # group open across >3 later allocations
                            pool = psB if jts is None else psST
                            tag = "pqkv" if jts is None else "st"
                            opart[(tt, ic)] = pool.tile(
                                [P, 512], f32, tag=tag,
                                name=f"po{tt}_{ic}")
                        po = opart[(tt, ic)]
                        for jt in range(lo, hi):
                            nc.tensor.matmul(
                                po[:],
                                yT_sb[:, jt, ts(tt, P)],
                                wo_sb[:, jt, ds(ic * 512, 512)],
                                start=(jt == 0), stop=(jt == JT_N - 1),
                            )
                        if hi < JT_N:
                            return
                        # f16 staging halves the store DMA; the partial is
                        # summed with the other core's on the host in f32
                        osb = osp.tile([P, 512], f16, tag="osb")
                        if ic == 0:
                            nc.scalar.copy(out=osb[:], in_=po[:])
                        else:
                            nc.vector.tensor_copy(out=osb[:], in_=po[:])
                        # tail stores avoid the Pool queue: its ~1us swdge
                        # descriptor gen would sit on the final barrier drain
                        if ic == 0:
                            eng = nc.scalar if tt >= 6 else nc.gpsimd
                        else:
                            eng = nc.sync
                        eng.dma_start(
                            out=out_v[:, tt, ds(ic * 512, 512)], in_=osb[:])

                    # ======== software-pipelined stage schedule =============
                    # stage g emits pair g's scores/exp chunks explicitly
                    # interleaved with later pairs' proj quanta and earlier
                    # pairs' AV quanta, so the PE always has filler between
                    # score chunks while ACT catches up on the exps. proj of
                    # pair g+1 runs during stage g; the out-proj is folded
                    # into stage 3 (on the freed proj-psum ring), gated per
                    # token-tile on pair-3's yT chunks.
                    def chunks_of(g):
                        return ([("sc", g, kt, 0) for kt in range(4)]
                                + [("sc", g, kt, 1) for kt in range(4)]
                                + [("sc", g, kt, 1) for kt in range(4, KT_N)])

                    def interleave(chunks, fillers):
                        seq = []
                        for i, c in enumerate(chunks):
                            seq.append(c)
                            if i < len(fillers):
                                seq.append(fillers[i])
                        return seq + fillers[len(chunks):]

                    for tt in range(TT_N):
                        emit_proj_tt(0, tt)

                    stages = {}
                    p = lambda g, lo, hi: [("proj", g, tt)
                                           for tt in range(lo, hi)]
                    a = lambda g: [("av", g, q2) for q2 in range(4)]
                    o = lambda tt, ic: ("out", tt, ic)
                    L = lambda i: [("ld", i)]
                    # proj fillers first (their rms/rotary/transpose chain
                    # gates the next stage); deferred-load dispatches late so
                    # they don't delay blends/rms in the Pool stream
                    stages[0] = interleave(
                        chunks_of(0), p(1, 0, 4) + L(0) + L(1) + p(1, 4, 8))
                    a0, a1 = a(0), a(1)
                    stages[1] = interleave(
                        chunks_of(1),
                        a0[0:2] + p(2, 0, 4) + L(2) + L(3) + p(2, 4, 8)
                        + a0[2:4])
                    stages[2] = interleave(
                        chunks_of(2),
                        a1[0:2] + p(3, 0, 4) + L(4) + p(3, 4, 8) + a1[2:4])
                    c3, a2q, a3q = chunks_of(3), a(2), a(3)
                    oP = lambda tt, ic: ("out", tt, ic, (0, 3))
                    oF = lambda tt, ic: ("out", tt, ic, (3, 4))
                    stages[3] = [c3[0], a2q[0], c3[1], a2q[1], c3[2], a2q[2],
                                 c3[3], a2q[3], c3[4], a3q[0], c3[5], a3q[1],
                                 o(0, 0), o(0, 1), c3[6], o(1, 0), o(1, 1),
                                 c3[7], o(2, 0), o(2, 1), c3[8], o(3, 0),
                                 c3[9], a3q[2], o(3, 1), c3[10], c3[11],
                                 a3q[3], oP(6, 0), oP(6, 1), o(4, 0),
                                 o(4, 1), o(5, 0), o(5, 1), oF(6, 0),
                                 oF(6, 1), o(7, 0), o(7, 1)]

                    for g in range(NG):
                        for quantum in stages[g]:
                            kind = quantum[0]
                            if kind == "sc":
                                emit_sc_chunk(quantum[1], quantum[2], quantum[3])
                            elif kind == "proj":
                                emit_proj_tt(quantum[1], quantum[2])
                            elif kind == "av":
                                emit_av_q2(quantum[1], quantum[2])
                            elif kind == "ld":
                                if _rep == 0:
                                    deferred_loads[quantum[1]]()
                            else:
                                emit_out(quantum[1], quantum[2],
                                         quantum[3] if len(quantum) > 3
                                         else None)

    split_sync_waits(nc)
    return nc


def make_core_inputs(x, qkvo_w, value_embeds, lambda_v):
    """Host-side prep: returns list of per-core input dicts."""
    import ml_dtypes

    F8NP = ml_dtypes.float8_e4m3

    x = np.asarray(x, dtype=np.float32)
    qkvo_w = np.asarray(qkvo_w, dtype=np.float32)
    value_embeds = np.asarray(value_embeds, dtype=np.float32)
    lambda_v = np.asarray(lambda_v, dtype=np.float32)

    def f8split(a):
        hi = a.astype(F8NP)
        lo = (a - hi.astype(np.float32)).astype(F8NP)
        return hi, lo

    freq = (1.0 / 1024.0) ** np.linspace(0.0, 1.0, DH // 4, dtype=np.float32)
    theta = np.arange(T, dtype=np.float32)[:, None] * freq[None, :]  # [T, 16]
    cos = np.cos(theta).astype(np.float32)
    sin = np.sin(theta).astype(np.float32)
    # [T, 128] tiled over all 8 (q|k, head, half) groups: cos repeats, sin
    # alternates sign; then re-tiled to [P, TT_N*128] (partition-contiguous)
    cos128 = np.concatenate([cos] * 8, axis=1).astype(np.float16)
    sin128 = np.concatenate([sin, -sin] * 4, axis=1).astype(np.float16)
    cosP = cos128.reshape(TT_N, P, P).transpose(1, 0, 2).reshape(P, TT_N * P)
    sinP = sin128.reshape(TT_N, P, P).transpose(1, 0, 2).reshape(P, TT_N * P)
    cosP = np.ascontiguousarray(cosP)
    sinP = np.ascontiguousarray(sinP)
    # additive causal ramp mask in fp8 double-row [64, 2, 128] layout
    # (j = i*64 + p): (Am.T @ Bm)[k, q] = -240 * max(0, k - q)
    pp = np.arange(64)[:, None, None]
    ii = np.arange(2)[None, :, None]
    mm = np.arange(P)[None, None, :]
    jj = ii * 64 + pp
    amask_np = (mm >= jj).astype(F8NP)
    bmask_np = (-240.0 * (jj > mm)).astype(F8NP)

    in_maps = []
    for c in range(N_CORES):
        b, hh = c // 2, c % 2
        R = slice(hh * H8 * DH, (hh + 1) * H8 * DH)
        wq = WSCALE * qkvo_w[0][R].T  # [D, 512]
        wk = WSCALE * qkvo_w[1][R].T
        wv = (WSCALE * lambda_v[0] * qkvo_w[2][R]).T
        # [D, NG, 384]: per pair the 128 q cols, 128 k cols, 128 v cols
        wqkv = np.empty((D, NG, 384), dtype=np.float32)
        for g in range(NG):
            wqkv[:, g, 0:128] = wq[:, g * 128 : (g + 1) * 128]
            wqkv[:, g, 128:256] = wk[:, g * 128 : (g + 1) * 128]
            wqkv[:, g, 256:384] = wv[:, g * 128 : (g + 1) * 128]
        w8h_np, w8l_np = f8split(wqkv)
        # block-transpose: xTt[tt*128+p, dt*128+j] = x[b][tt*128+j, dt*128+p]
        xb = x[b].reshape(TT_N, P, DT_N, P)
        xTt = np.ascontiguousarray(xb.transpose(0, 3, 2, 1).reshape(T, D))
        x8h_np, x8l_np = f8split(xTt)
        in_maps.append({
            "x8h": x8h_np,
            "x8l": x8l_np,
            "w8h": w8h_np,
            "w8l": w8l_np,
            "woT": np.ascontiguousarray(
                qkvo_w[3][:, R].T / WSCALE).astype(np.float16),
            "ve": (WSCALE * lambda_v[1] * value_embeds[:T, R]).astype(
                np.float16),
            "cosd": cosP,
            "sind": sinP,
            "amask": amask_np,
            "bmask": bmask_np,
            "ident": np.eye(P, dtype=np.float16),
        })
    return in_maps


_NC_CACHE = {}


def _get_nc(reps=1):
    if reps not in _NC_CACHE:
        _NC_CACHE[reps] = build_nc(reps)
    return _NC_CACHE[reps]


def kernel(x, qkvo_w, value_embeds, lambda_v):
    from concourse.bass_utils import run_bass_kernel_spmd

    nc = _get_nc()
    in_maps = make_core_inputs(x, qkvo_w, value_embeds, lambda_v)
    res = run_bass_kernel_spmd(nc, in_maps, list(range(N_CORES))).results
    out = np.empty((B, T, D), dtype=np.float32)
    for b in range(B):
        out[b] = (res[2 * b]["out"].astype(np.float32)
                  + res[2 * b + 1]["out"].astype(np.float32))
    return out

